# revision 60
# baseline (speedup 1.0000x reference)
"""Trainium2 Bass kernel for nn_CA_SA_v4 (dense transformer, 8 NeuronCores).

Sharding: core i handles batch b=i//4 and query-token slab q=i%4 (1024 of 4096
tokens). The 3x3 conv stack is row-sharded (16 rows + halo per core); the raw
style feature Fs is AllGathered in bf16 per batch group in two token-half
waves, preceded by a tiny stats AllGather so normalization constants arrive
early and wave B streams in behind wave-A compute. Attention algebra is folded
on the host: S uses W_g^T W_f (no G conv) and R uses W_o W_h (no o convs);
per-query softmax constants cancel, the per-key bias rides as column 256 of
the value projection, and the style-mean offset becomes a per-channel constant
folded into the residual bias. The final mvn rescale is folded into the output
conv weights. All matmuls bf16 with fp32 PSUM accumulation.
"""
import sys

sys.path.insert(0, "/opt/trn_rl_repo")

import os
import tempfile

# The deployment's NEFF cache keys on the outer HLO signature only (the
# embedded BIR is ignored), so two different bass programs with identical
# I/O shapes collide. Use a private cache dir per process.
os.environ["NEURON_COMPILE_CACHE_URL"] = tempfile.mkdtemp(prefix="neff_cache_")

import numpy as np
import ml_dtypes

import concourse.mybir as mybir
import concourse.tile as tile
import concourse.bacc as bacc
from concourse.bass_utils import run_bass_kernel_spmd

F32 = mybir.dt.float32
BF16 = mybir.dt.bfloat16
FP8E4 = mybir.dt.float8e4
FP8E5 = mybir.dt.float8e5
DR = mybir.MatmulPerfMode.DoubleRow
AL = mybir.AluOpType
AF = mybir.ActivationFunctionType
AX = mybir.AxisListType

# exp shift constants per attention (softmax-invariant; keep exp(S - c) inside
# fp8e5 range: S1 max ~10.6, S2 max ~19.6 on this input distribution)
C_SHIFT = {1: 2.0, 2: 10.0}

B, C, CLIP, H, W = 2, 256, 512, 64, 64
N = H * W              # 4096 tokens
NSLAB = N // 4         # 1024 query tokens per core
ROWS = 16              # output rows per core
NCORES = 8
GROUPS = [[0, 1, 2, 3], [4, 5, 6, 7]]
EPS = 1e-5
VAR_CORR = float(N) / float(N - 1)  # ddof=1 correction

# awt rows
R_V1, R_V2, R_H1, R_H2, R_OUT = range(5)
# biasv rows
B_FS1, B_FS2, B_RB, B_OUT = range(4)

# m-tile waves matching the two style AllGather halves: wave A covers the
# first 512 tokens of every slab, wave B the second 512. The attention loop
# processes key tiles in PAIRS (DoubleRow fp8 contraction over 2x128 keys).
WAVE_A = [t * 8 + j for t in range(4) for j in range(4)]
WAVE_B = [t * 8 + j for t in range(4) for j in range(4, 8)]
PAIRS_A = [t * 8 + j for t in range(4) for j in (0, 2)]
PAIRS_B = [t * 8 + j for t in range(4) for j in (4, 6)]

_CACHE: dict = {}


def _build():
    nc = bacc.Bacc("TRN2", num_devices=NCORES, debug=False, target_bir_lowering=False)

    xclip_d = nc.dram_tensor("xclip", [128, 2, 2, 20, 68], FP8E4, kind="ExternalInput").ap()
    xcont_d = nc.dram_tensor("xcont", [128, 2, 4096], F32, kind="ExternalInput").ap()
    mask_d = nc.dram_tensor("maskio", [128, 18, 1], BF16, kind="ExternalInput").ap()
    w1t_d = nc.dram_tensor("w1t", [128, 2, 2, 9, 256], FP8E4, kind="ExternalInput").ap()
    w2t_d = nc.dram_tensor("w2t", [128, 2, 9, 256], BF16, kind="ExternalInput").ap()
    awt_d = nc.dram_tensor("awt", [128, 2, 5, 257], BF16, kind="ExternalInput").ap()
    biasv_d = nc.dram_tensor("biasv", [128, 2, 4], F32, kind="ExternalInput").ap()
    out_d = nc.dram_tensor("out", [256, NSLAB], F32, kind="ExternalOutput").ap()

    with tile.TileContext(nc) as tc:
        _body(nc, tc, xclip_d, xcont_d, mask_d, w1t_d, w2t_d, awt_d, biasv_d, out_d)
    nc.compile()
    return nc


def _body(nc, tc, xclip_d, xcont_d, mask_d, w1t_d, w2t_d, awt_d, biasv_d, out_d):
    from contextlib import ExitStack

    ctx = ExitStack()
    const = ctx.enter_context(tc.tile_pool(name="const", bufs=1))
    stats = ctx.enter_context(tc.tile_pool(name="stats", bufs=1))
    dram = ctx.enter_context(tc.tile_pool(name="dram", bufs=1, space="DRAM"))

    biasv = const.tile([128, 2, 4], F32)
    mask = const.tile([128, 18, 1], BF16)
    awt = const.tile([128, 2, 5, 257], BF16)
    eps_t = const.tile([128, 1], F32)

    # psw: conv accumulators, fq2 staging, k1 (closed before attention)
    psw_cm = tc.tile_pool(name="psw", bufs=2, space="PSUM")
    psw = psw_cm.__enter__()

    # fc pool outlives the conv pools -> allocate before them (LIFO stacks)
    fcp = ctx.enter_context(tc.tile_pool(name="fc", bufs=1))
    f_c = fcp.tile([128, 2, 1024], F32)
    f_c_bf = fcp.tile([128, 2, 1024], BF16)

    # ================= conv stack (row slab, with halo) =================
    convp_cm = tc.tile_pool(name="convio", bufs=1)
    convp = convp_cm.__enter__()
    w1t = convp.tile([128, 2, 2, 9, 256], FP8E4)
    xclip = convp.tile([128, 2, 2, 20, 68], FP8E4)
    w2t = convp.tile([128, 2, 9, 256], BF16)
    y1 = convp.tile([128, 2, 18, 66], BF16)
    nc.gpsimd.dma_start(biasv[:], biasv_d)
    # w1t in kt-pair chunks + xclip in row chunks so conv1 starts early
    nc.gpsimd.dma_start(w1t[:, 0:1], w1t_d[:, 0:1])
    nc.gpsimd.dma_start(xclip[:, :, :, 0:8], xclip_d[:, :, :, 0:8])
    for rs, re in ((8, 14), (14, 20)):
        nc.gpsimd.dma_start(xclip[:, :, :, rs:re], xclip_d[:, :, :, rs:re])
    nc.gpsimd.dma_start(w1t[:, 1:2], w1t_d[:, 1:2])
    nc.vector.memset(y1[:], 0.0)
    nc.vector.memset(eps_t[:], EPS)

    contp_cm = tc.tile_pool(name="cont", bufs=1)
    contp = contp_cm.__enter__()
    xcont = contp.tile([128, 2, 4096], F32)
    # xcont early: the content-mvn chain gates the query projections
    nc.gpsimd.dma_start(xcont[:], xcont_d)
    nc.gpsimd.dma_start(mask[:], mask_d)
    nc.gpsimd.dma_start(w2t[:], w2t_d)
    nc.gpsimd.dma_start(awt[:], awt_d)

    sqp_cm = tc.tile_pool(name="sqscr", bufs=1)
    sqp = sqp_cm.__enter__()
    sq = sqp.tile([128, 4096], F32)

    # content stats (DVE; overlaps conv1 on PE)
    c_mean = stats.tile([128, 2], F32)
    c_nmrs = stats.tile([128, 2], F32)  # -mean/std
    c_rstd = stats.tile([128, 2], F32)
    tmp_a = stats.tile([128, 2], F32)
    tmp_b = stats.tile([128, 2], F32)
    for oc in range(2):
        nc.vector.reduce_sum(tmp_a[:, oc : oc + 1], xcont[:, oc], axis=AX.X)
        nc.vector.scalar_tensor_tensor(
            out=sq[:],
            in0=xcont[:, oc],
            scalar=1.0,
            in1=xcont[:, oc],
            op0=AL.mult,
            op1=AL.mult,
            accum_out=tmp_b[:, oc : oc + 1],
        )
    nc.vector.tensor_scalar_mul(c_mean[:], tmp_a[:], 1.0 / N)
    nc.vector.tensor_scalar_mul(tmp_b[:], tmp_b[:], 1.0 / N)  # E[x^2]
    nc.vector.tensor_mul(tmp_a[:], c_mean[:], c_mean[:])
    nc.vector.tensor_sub(tmp_b[:], tmp_b[:], tmp_a[:])  # biased var
    nc.scalar.activation(tmp_b[:], tmp_b[:], AF.Sqrt, bias=eps_t[:], scale=VAR_CORR)
    nc.vector.reciprocal(c_rstd[:], tmp_b[:])
    nc.vector.tensor_mul(c_nmrs[:], c_mean[:], c_rstd[:])
    nc.vector.tensor_scalar_mul(c_nmrs[:], c_nmrs[:], -1.0)

    # conv1: 512 -> 256, 18 output rows (16 + halo), relu. fp8 DoubleRow over
    # kt pairs (weights host-scaled x16; undone by the activation scale).
    for rb in range(3):  # row blocks of 6
        for oc in range(2):
            pc = psw.tile([128, 1024], F32, tag="work", name=f"pc1_{rb}_{oc}")
            first = True
            for ktp in range(2):
                for off in range(9):
                    di, dj = off // 3, off % 3
                    nc.tensor.matmul(
                        pc[:, 0:384],
                        w1t[:, ktp, :, off, oc * 128 : (oc + 1) * 128],
                        xclip[:, ktp, :, rb * 6 + di : rb * 6 + di + 6, dj : dj + 64],
                        perf_mode=DR,
                        start=first,
                        stop=(off == 8 and ktp == 1),
                    )
                    first = False
            nc.scalar.activation(
                y1[:, oc, rb * 6 : rb * 6 + 6, 1:65],
                pc[:, 0:384].rearrange("p (r w) -> p r w", r=6),
                AF.Relu,
                bias=biasv[:, oc, B_FS1 : B_FS1 + 1],
                scale=1.0 / 16,
            )
    # zero the halo rows that lie outside the image (per-core mask data)
    for oc in range(2):
        nc.vector.tensor_mul(y1[:, oc], y1[:, oc], mask[:].to_broadcast((128, 18, 66)))

    # f_c (content slab mvn) — frees xcont early
    for oc in range(2):
        nc.scalar.activation(
            f_c[:, oc],
            xcont[:, oc, 0:NSLAB],
            AF.Identity,
            bias=c_nmrs[:, oc : oc + 1],
            scale=c_rstd[:, oc : oc + 1],
        )
    nc.vector.tensor_copy(f_c_bf[:], f_c[:])

    # staging pool for the conv2 output slab (attention tiles come later,
    # after the conv pools are released — SBUF is tight here)
    stylep_cm = tc.tile_pool(name="style", bufs=1)
    stylep = stylep_cm.__enter__()
    fs_bf = stylep.tile([128, 2, 1024], BF16)   # local slab, raw Fs

    # conv2: 256 -> 256, 16 output rows -> raw Fs slab (bf16). Stat partials
    # come from a biased f32 copy of each row-block's PSUM so they overlap
    # the next block's matmuls instead of gating the stats AllGather.
    pstat4 = stats.tile([128, 2, 2, 2], F32)  # (rb, oc, {sum, sumsq})
    for rb in range(2):  # row blocks of 8
        for oc in range(2):
            pc = psw.tile([128, 1024], F32, tag="work", name=f"pc2_{rb}_{oc}")
            first = True
            for off in range(9):
                di, dj = off // 3, off % 3
                for kt in range(2):
                    nc.tensor.matmul(
                        pc[:, 0:512],
                        w2t[:, kt, off, oc * 128 : (oc + 1) * 128],
                        y1[:, kt, rb * 8 + di : rb * 8 + di + 8, dj : dj + 64],
                        start=first,
                        stop=(off == 8 and kt == 1),
                    )
                    first = False
            nc.scalar.activation(
                fs_bf[:, oc, rb * 512 : (rb + 1) * 512],
                pc[:, 0:512],
                AF.Identity,
                bias=biasv[:, oc, B_FS2 : B_FS2 + 1],
            )
            nc.vector.tensor_scalar_add(
                sq[:, 0:512], pc[:, 0:512], biasv[:, oc, B_FS2 : B_FS2 + 1]
            )
            nc.vector.reduce_sum(pstat4[:, rb, oc, 0:1], sq[:, 0:512], axis=AX.X)
            nc.vector.scalar_tensor_tensor(
                out=sq[:, 512:1024],
                in0=sq[:, 0:512],
                scalar=1.0,
                in1=sq[:, 0:512],
                op0=AL.mult,
                op1=AL.mult,
                accum_out=pstat4[:, rb, oc, 1:2],
            )

    pstat = stats.tile([128, 2, 2], F32)  # (oc, {sum, sumsq})
    nc.vector.tensor_add(
        pstat[:].rearrange("p oc s -> p (oc s)"),
        pstat4[:, 0].rearrange("p oc s -> p (oc s)"),
        pstat4[:, 1].rearrange("p oc s -> p (oc s)"),
    )

    # wave A (ready after conv2 row-block 0) goes first on the CC stream,
    # then the tiny stats gather, then wave B.
    ag_st_in = dram.tile([1, 512], F32)
    ag_st_out = dram.tile([4, 512], F32)
    ag_fs_in = {w: dram.tile([256, 512], BF16, name=f"agfsi{w}") for w in range(2)}
    ag_fs_out = {w: dram.tile([1024, 512], BF16, name=f"agfso{w}") for w in range(2)}
    # The tiny stats gather goes FIRST on the CC stream: it gates all
    # attention prep AND absorbs the first-collective warmup cost that would
    # otherwise inflate the big wave-A gather. Both fs waves stage their oc=1
    # rows through scratches copied (on the DVE queue) after the stats pack,
    # so the scheduler cannot reorder them ahead of the stats collective.
    nc.gpsimd.dma_start(
        ag_st_in[0, :].rearrange("(oc p s) -> p oc s", p=128, s=2), pstat[:]
    )
    arow = stylep.tile([128, 512], BF16)
    brow = stylep.tile([128, 512], BF16)
    nc.vector.tensor_copy(arow[:], fs_bf[:, 1, 0:512])
    nc.vector.tensor_copy(brow[:], fs_bf[:, 1, 512:1024])
    nc.gpsimd.dma_start(ag_fs_in[0][0:128, :], fs_bf[:, 0, 0:512])
    nc.gpsimd.dma_start(ag_fs_in[0][128:256, :], arow[:])
    nc.gpsimd.dma_start(ag_fs_in[1][0:128, :], fs_bf[:, 0, 512:1024])
    nc.gpsimd.dma_start(ag_fs_in[1][128:256, :], brow[:])
    nc.gpsimd.collective_compute(
        "AllGather", AL.bypass, replica_groups=GROUPS,
        ins=[ag_st_in.opt()], outs=[ag_st_out.opt()],
    )
    nc.gpsimd.collective_compute(
        "AllGather", AL.bypass, replica_groups=GROUPS,
        ins=[ag_fs_in[0].opt()], outs=[ag_fs_out[0].opt()],
    )
    nc.gpsimd.collective_compute(
        "AllGather", AL.bypass, replica_groups=GROUPS,
        ins=[ag_fs_in[1].opt()], outs=[ag_fs_out[1].opt()],
    )

    stylep_cm.__exit__(None, None, None)
    sqp_cm.__exit__(None, None, None)
    contp_cm.__exit__(None, None, None)
    convp_cm.__exit__(None, None, None)

    # attention pool (allocated once the conv pools are released)
    # fp8 scaling scheme (all exact, compensated downstream):
    #   fs8 = fs/4, fq1_8 = 4*fq1/sigma, fq2_8 = 8*fq2, fs28 = fs2/8,
    #   awt_h1s8 = 64*Woh1/sigma, awt_h28 = 64*Woh2, ht8 = 8*ht_true,
    #   ones8 = 8 => R/rowsum ratio exact.
    attnp = ctx.enter_context(tc.tile_pool(name="attn", bufs=1))
    fs_full = attnp.tile([128, 2, 4096], BF16)  # raw Fs, all tokens
    fs8 = attnp.tile([128, 2, 4096], FP8E4)     # fs/4 (S1/ht1 lhsT)
    fs28 = attnp.tile([128, 2, 4096], FP8E4)    # fs2/8 (S2/ht2 lhsT)
    fq8 = {a: attnp.tile([128, 2, 1024], FP8E4, name=f"fq8_{a}") for a in (1, 2)}
    ht8 = {a: attnp.tile([128, 16, 2, 272], FP8E4, name=f"ht8_{a}") for a in (1, 2)}
    betv = {a: attnp.tile([128, 32], BF16, name=f"betv{a}") for a in (1, 2)}
    et_h18 = {a: attnp.tile([128, 8, 2, 512], FP8E5, name=f"eth1_{a}") for a in (1, 2)}
    awt_h1s8 = attnp.tile([128, 2, 257], FP8E4)
    awt_h1sb = attnp.tile([128, 2, 257], BF16)  # bf16 copy for the k1 matmul
    awt_h28 = attnp.tile([128, 2, 257], FP8E4)
    ones8 = attnp.tile([128, 2, 16], FP8E4)
    awt_outs = attnp.tile([128, 2, 256], BF16)  # row R_OUT scaled by alpha
    rbf = attnp.tile([128, 2, 1024], BF16)      # residual sum r (bf16)
    rbc = attnp.tile([128, 2, 512], F32)        # 1/rowsum broadcast (per a)
    rs_sb = attnp.tile([128, 2, 512], F32)      # rowsum staging (p0 only)
    ep_t1 = attnp.tile([128, 512], F32)
    ep_t2 = attnp.tile([128, 512], F32)
    ep_sq = attnp.tile([128, 512], F32)
    osb = attnp.tile([128, 2, 1024], F32)

    nc.vector.memset(ones8[:], 8.0)
    for kt in range(2):
        nc.vector.tensor_scalar_mul(awt_h28[:, kt], awt[:, kt, R_H2, :], 64.0)
    cshift = attnp.tile([128, 2], F32)
    nc.vector.memset(cshift[:, 0:1], -C_SHIFT[1])
    nc.vector.memset(cshift[:, 1:2], -C_SHIFT[2])

    # unpack each AllGather half into fs_full as it lands. Wave-A unpacks go
    # on the Sync queue, wave B on GpSimd — one FIFO can't head-block the
    # other wave's data.
    for w in range(2):
        eng = nc.sync if w == 0 else nc.gpsimd
        for t in range(4):
            eng.dma_start(
                fs_full[:, :, t * 1024 + w * 512 : t * 1024 + (w + 1) * 512],
                ag_fs_out[w][t * 256 : (t + 1) * 256, :].rearrange(
                    "(oc p) n -> p oc n", p=128
                ),
            )

    # ---- query projections (overlap the AllGathers) ----
    # fq2 needs no style stats; copied out of PSUM immediately.
    for oc in range(2):
        pq = psw.tile([128, 1024], F32, tag="work", name=f"pq2_{oc}")
        for half in range(2):
            for kt in range(2):
                nc.tensor.matmul(
                    pq[:, half * 512 : (half + 1) * 512],
                    awt[:, kt, R_V2, oc * 128 : (oc + 1) * 128],
                    f_c_bf[:, kt, half * 512 : (half + 1) * 512],
                    start=(kt == 0),
                    stop=(kt == 1),
                )
        nc.scalar.activation(fq8[2][:, oc], pq[:], AF.Identity, scale=8.0)
    # fq1 held in PSUM until 1/sigma arrives
    pq1_cm = tc.tile_pool(name="pq1", bufs=1, space="PSUM")
    pq1p = pq1_cm.__enter__()
    pq1 = {}
    for oc in range(2):
        pq = pq1p.tile([128, 1024], F32, tag=f"pq1_{oc}", name=f"pq1_{oc}")
        for half in range(2):
            for kt in range(2):
                nc.tensor.matmul(
                    pq[:, half * 512 : (half + 1) * 512],
                    awt[:, kt, R_V1, oc * 128 : (oc + 1) * 128],
                    f_c_bf[:, kt, half * 512 : (half + 1) * 512],
                    start=(kt == 0),
                    stop=(kt == 1),
                )
        pq1[oc] = pq

    # ---- style stats arrive (tiny AG; unpack on the Vector queue, which
    # consumes them immediately after) ----
    sraw = stats.tile([128, 4, 2, 2], F32)  # (t, oc, s)
    nc.scalar.dma_start(
        sraw[:],
        ag_st_out[:, :].rearrange("t (oc p s) -> p t oc s", p=128, s=2),
    )
    s_mean = stats.tile([128, 2], F32)
    s_std = stats.tile([128, 2], F32)
    s_rstd = stats.tile([128, 2], F32)
    s_nmrs = stats.tile([128, 2], F32)
    mu_bf = stats.tile([128, 2], BF16)
    stot = stats.tile([128, 2, 2], F32)  # (oc, s)
    nc.vector.reduce_sum(
        stot[:].rearrange("p oc s -> p (oc s)"),
        sraw[:].rearrange("p t oc s -> p (oc s) t"),
        axis=AX.X,
    )
    nc.vector.tensor_scalar_mul(s_mean[:], stot[:, :, 0], 1.0 / N)
    nc.vector.tensor_scalar_mul(tmp_b[:], stot[:, :, 1], 1.0 / N)
    nc.vector.tensor_mul(tmp_a[:], s_mean[:], s_mean[:])
    nc.vector.tensor_sub(tmp_b[:], tmp_b[:], tmp_a[:])
    nc.scalar.activation(s_std[:], tmp_b[:], AF.Sqrt, bias=eps_t[:], scale=VAR_CORR)
    nc.vector.reciprocal(s_rstd[:], s_std[:])
    nc.vector.tensor_mul(s_nmrs[:], s_mean[:], s_rstd[:])
    nc.vector.tensor_scalar_mul(s_nmrs[:], s_nmrs[:], -1.0)
    nc.vector.tensor_copy(mu_bf[:], s_mean[:])

    # scale value projections for a1 by 1/sigma (per style channel)
    s_rstd4 = stats.tile([128, 2], F32)
    s_rstd64 = stats.tile([128, 2], F32)
    nc.vector.tensor_scalar_mul(s_rstd4[:], s_rstd[:], 4.0)
    nc.vector.tensor_scalar_mul(s_rstd64[:], s_rstd[:], 64.0)
    for kt in range(2):
        nc.vector.tensor_scalar_mul(
            awt_h1s8[:, kt], awt[:, kt, R_H1, :], s_rstd64[:, kt : kt + 1]
        )
        nc.vector.tensor_scalar_mul(
            awt_h1sb[:, kt], awt[:, kt, R_H1, :], s_rstd[:, kt : kt + 1]
        )
    # k1[o] = sum_c Woh1'[o,c] * mu_c  (mean offset of folded a1 values)
    pk = psw.tile([128, 2], F32, tag="work", name="pk1")
    for oc in range(2):
        for kt in range(2):
            nc.tensor.matmul(
                pk[:, oc : oc + 1],
                awt_h1sb[:, kt, oc * 128 : (oc + 1) * 128],
                mu_bf[:, kt : kt + 1],
                start=(kt == 0),
                stop=(kt == 1),
            )
    rbias_rt = stats.tile([128, 2], F32)
    nc.vector.tensor_sub(rbias_rt[:], biasv[:, :, B_RB], pk[:])

    # fq1: copy out of PSUM with 4/sigma fold (fp8)
    for oc in range(2):
        nc.scalar.activation(
            fq8[1][:, oc], pq1[oc][:], AF.Identity, scale=s_rstd4[:, oc : oc + 1]
        )
    pq1_cm.__exit__(None, None, None)
    psw_cm.__exit__(None, None, None)

    # per-wave fp8 conversions (DVE; chunk order matches the m-loop)
    def fs8_wave(w):
        for t in range(4):
            for oc in range(2):
                cols = slice(t * 1024 + w * 512, t * 1024 + (w + 1) * 512)
                nc.vector.tensor_scalar_mul(
                    fs8[:, oc, cols], fs_full[:, oc, cols], 0.25
                )

    def fs2_wave(w):
        for t in range(4):
            for oc in range(2):
                cols = slice(t * 1024 + w * 512, t * 1024 + (w + 1) * 512)
                nc.vector.tensor_scalar(
                    out=ep_sq[:],
                    in0=fs_full[:, oc, cols],
                    scalar1=s_rstd[:, oc : oc + 1],
                    scalar2=s_nmrs[:, oc : oc + 1],
                    op0=AL.mult,
                    op1=AL.add,
                )
                nc.vector.scalar_tensor_tensor(
                    out=fs28[:, oc, cols],
                    in0=ep_sq[:],
                    scalar=0.125,
                    in1=ep_sq[:],
                    op0=AL.mult,
                    op1=AL.mult,
                )

    # attention PSUM pools: 6 banks accumulators + 2 banks S/ht staging
    psS_cm = tc.tile_pool(name="psS", bufs=2, space="PSUM")
    psS = psS_cm.__enter__()
    psR_cm = tc.tile_pool(name="psR", bufs=1, space="PSUM")
    psR = psR_cm.__enter__()
    etp = ctx.enter_context(tc.tile_pool(name="etp", bufs=3))

    def ht_tile(a, mt):
        ph = psS.tile([128, 512], F32, tag="st", name=f"ph{a}_{mt}")
        src8 = fs8 if a == 1 else fs28
        nc.tensor.matmul(
            ph[:, 0:257],
            src8[:, :, mt * 128 : (mt + 1) * 128],
            (awt_h1s8 if a == 1 else awt_h28)[:],
            perf_mode=DR,
        )
        nc.scalar.activation(
            ht8[a][:, mt // 2, mt % 2, 0:256],
            ph[:, 0:256],
            AF.Identity,
            scale=(0.5 if a == 1 else 1.0),
        )
        nc.scalar.activation(
            betv[a][:, mt : mt + 1],
            ph[:, 256:257],
            AF.Identity,
            scale=(1.0 / 16 if a == 1 else 1.0 / 8),
            bias=cshift[:, a - 1 : a],
        )

    fs8_wave(0)
    fs2_wave(0)
    for mt in WAVE_A:
        ht_tile(1, mt)
        ht_tile(2, mt)

    ep_acc = stats.tile([128, 2, 2, 2], F32)  # (half, oc, {sum, sumsq})

    def s_tile(a, mt, half, out_et):
        st = psS.tile([128, 512], F32, tag="st", name=f"st{a}_{mt}_{half}")
        src8 = fs8 if a == 1 else fs28
        nc.tensor.matmul(
            st[:],
            src8[:, :, mt * 128 : (mt + 1) * 128],
            fq8[a][:, :, half * 512 : (half + 1) * 512],
            perf_mode=DR,
        )
        nc.scalar.activation(out_et, st[:], AF.Exp, bias=betv[a][:, mt : mt + 1])

    rps = {}
    rsum = {}

    def attn_pair(a, mt, ppos, e8):
        # rowsum first: the final pair's reciprocal starts before the
        # trailing R matmuls retire
        nc.tensor.matmul(
            rsum[a][:],
            ones8[:, :, 0:1],
            e8,
            perf_mode=DR,
            start=(ppos == 0),
            stop=(ppos == 15),
        )
        for oc in range(2):
            nc.tensor.matmul(
                rps[a][oc][:],
                ht8[a][:, mt // 2, :, oc * 128 : (oc + 1) * 128],
                e8,
                perf_mode=DR,
                start=(ppos == 0),
                stop=(ppos == 15),
            )

    rn2 = attnp.tile([128, 2, 512], F32)

    def epilogue_a2(half):
        # a2's normalization runs while a1's matmuls still stream
        nc.vector.reciprocal(rs_sb[0:1, 1], rsum[2][:])
        nc.gpsimd.partition_broadcast(rbc[:, 1], rs_sb[0:1, 1])
        for oc in range(2):
            nc.vector.tensor_mul(rn2[:, oc], rps[2][oc][:], rbc[:, 1])
            nc.vector.tensor_scalar_add(
                rn2[:, oc], rn2[:, oc], rbias_rt[:, oc : oc + 1]
            )

    def epilogue_a1(half):
        nc.vector.reciprocal(rs_sb[0:1, 0], rsum[1][:])
        nc.gpsimd.partition_broadcast(rbc[:, 0], rs_sb[0:1, 0])
        cols = slice(half * 512, (half + 1) * 512)
        for oc in range(2):
            nc.vector.tensor_mul(ep_t1[:], rps[1][oc][:], rbc[:, 0])
            nc.vector.tensor_add(ep_t1[:], ep_t1[:], f_c[:, oc, cols])
            nc.vector.scalar_tensor_tensor(
                out=rbf[:, oc, cols],
                in0=ep_t1[:],
                scalar=0.0,
                in1=rn2[:, oc],
                op0=AL.add,
                op1=AL.add,
                accum_out=ep_acc[:, half, oc, 0:1],
            )
            nc.vector.scalar_tensor_tensor(
                out=ep_sq[:],
                in0=rbf[:, oc, cols],
                scalar=1.0,
                in1=rbf[:, oc, cols],
                op0=AL.mult,
                op1=AL.mult,
                accum_out=ep_acc[:, half, oc, 1:2],
            )

    def epilogue(half):
        for a in (1, 2):
            nc.vector.reciprocal(rs_sb[0:1, a - 1], rsum[a][:])
            nc.gpsimd.partition_broadcast(rbc[:, a - 1], rs_sb[0:1, a - 1])
        cols = slice(half * 512, (half + 1) * 512)
        for oc in range(2):
            # r = (rps1/rs1 + f_c) + (rps2/rs2 + rbias_rt), bf16, fused stats
            nc.vector.tensor_mul(ep_t1[:], rps[1][oc][:], rbc[:, 0])
            nc.vector.tensor_add(ep_t1[:], ep_t1[:], f_c[:, oc, cols])
            nc.vector.tensor_mul(ep_t2[:], rps[2][oc][:], rbc[:, 1])
            nc.vector.scalar_tensor_tensor(
                out=rbf[:, oc, cols],
                in0=ep_t2[:],
                scalar=rbias_rt[:, oc : oc + 1],
                in1=ep_t1[:],
                op0=AL.add,
                op1=AL.add,
                accum_out=ep_acc[:, half, oc, 0:1],
            )
            nc.vector.scalar_tensor_tensor(
                out=ep_sq[:],
                in0=rbf[:, oc, cols],
                scalar=1.0,
                in1=rbf[:, oc, cols],
                op0=AL.mult,
                op1=AL.mult,
                accum_out=ep_acc[:, half, oc, 1:2],
            )

    def run_seq(items, half, tagsfx):
        """Software-pipelined S->exp->R over key-tile pairs: emit the S pair
        for step i+1 before R for step i so the PE never head-blocks on the
        exp."""
        pend = None
        for ppos, mt, a in items:
            e8 = etp.tile(
                [128, 2, 512], FP8E5, tag="et", name=f"et{a}_{mt}_{tagsfx}"
            )
            s_tile(a, mt, half, e8[:, 0])
            s_tile(a, mt + 1, half, e8[:, 1])
            if pend is not None:
                attn_pair(*pend)
            pend = (a, mt, ppos, e8[:])
        attn_pair(*pend)

    # ---- half 0: wave A, precache h1 S on wave A, then wave B ----
    rps = {
        a: [psR.tile([128, 512], F32, tag=f"r{a}_{oc}", name=f"r{a}_{oc}_h0")
            for oc in range(2)]
        for a in (1, 2)
    }
    rsum = {a: psR.tile([1, 512], F32, tag=f"rs{a}", name=f"rsum{a}_h0") for a in (1, 2)}
    run_seq([(p, mt, a) for p, mt in enumerate(PAIRS_A) for a in (1, 2)], 0, "h0a")
    # h1 S precache for wave A (fills the AG-B window)
    for i, mt in enumerate(PAIRS_A):
        for a in (1, 2):
            s_tile(a, mt, 1, et_h18[a][:, i, 0])
            s_tile(a, mt + 1, 1, et_h18[a][:, i, 1])
    # wave B prep + compute
    fs8_wave(1)
    fs2_wave(1)
    for mt in WAVE_B:
        ht_tile(1, mt)
        ht_tile(2, mt)
    run_seq(
        [(8 + p, mt, a) for p, mt in enumerate(PAIRS_B) for a in (1, 2)], 0, "h0b"
    )
    epilogue(0)

    # ---- half 1: wave A cached, wave B fresh ----
    rps = {
        a: [psR.tile([128, 512], F32, tag=f"r{a}_{oc}", name=f"r{a}_{oc}_h1")
            for oc in range(2)]
        for a in (1, 2)
    }
    rsum = {a: psR.tile([1, 512], F32, tag=f"rs{a}", name=f"rsum{a}_h1") for a in (1, 2)}
    # a2 fully first, then a1: a2's epilogue chain hides under a1's matmuls
    for a in (2, 1):
        for ppos, mt in enumerate(PAIRS_A):
            attn_pair(a, mt, ppos, et_h18[a][:, ppos])
        run_seq(
            [(8 + p, mt, a) for p, mt in enumerate(PAIRS_B)], 1, f"h1b{a}"
        )
        if a == 2:
            epilogue_a2(1)
    epilogue_a1(1)

    psR_cm.__exit__(None, None, None)
    psS_cm.__exit__(None, None, None)

    # ---- r stats -> AllGather #2 -> final mvn rescale (folded into W) ----
    rstat = stats.tile([128, 2, 2], F32)
    nc.vector.tensor_add(
        rstat[:].rearrange("p oc s -> p (oc s)"),
        ep_acc[:, 0].rearrange("p oc s -> p (oc s)"),
        ep_acc[:, 1].rearrange("p oc s -> p (oc s)"),
    )
    ag2_in = dram.tile([1, 512], F32)
    ag2_out = dram.tile([4, 512], F32)
    nc.gpsimd.dma_start(
        ag2_in[0, :].rearrange("(oc p s) -> p oc s", p=128, s=2), rstat[:]
    )
    nc.gpsimd.collective_compute(
        "AllGather", AL.bypass, replica_groups=GROUPS,
        ins=[ag2_in.opt()], outs=[ag2_out.opt()],
    )
    rraw = stats.tile([128, 4, 2, 2], F32)
    nc.gpsimd.dma_start(
        rraw[:],
        ag2_out[:, :].rearrange("t (oc p s) -> p t oc s", p=128, s=2),
    )
    rtot = stats.tile([128, 2, 2], F32)
    nc.vector.reduce_sum(
        rtot[:].rearrange("p oc s -> p (oc s)"),
        rraw[:].rearrange("p t oc s -> p (oc s) t"),
        axis=AX.X,
    )
    alpha = stats.tile([128, 2], F32)
    betav = stats.tile([128, 2], F32)
    beta_bf = stats.tile([128, 2], BF16)
    r_mean = stats.tile([128, 2], F32)
    nc.vector.tensor_scalar_mul(r_mean[:], rtot[:, :, 0], 1.0 / N)
    nc.vector.tensor_scalar_mul(tmp_b[:], rtot[:, :, 1], 1.0 / N)
    nc.vector.tensor_mul(tmp_a[:], r_mean[:], r_mean[:])
    nc.vector.tensor_sub(tmp_b[:], tmp_b[:], tmp_a[:])
    nc.scalar.activation(tmp_b[:], tmp_b[:], AF.Sqrt, bias=eps_t[:], scale=VAR_CORR)
    nc.vector.reciprocal(tmp_b[:], tmp_b[:])  # 1/std_r
    nc.vector.tensor_mul(alpha[:], s_std[:], tmp_b[:])
    nc.vector.tensor_mul(tmp_a[:], r_mean[:], alpha[:])
    nc.vector.tensor_sub(betav[:], s_mean[:], tmp_a[:])
    nc.vector.tensor_copy(beta_bf[:], betav[:])

    # fold alpha into the final conv weights; beta into its bias
    for kt in range(2):
        nc.vector.tensor_scalar_mul(
            awt_outs[:, kt], awt[:, kt, R_OUT, 0:256], alpha[:, kt : kt + 1]
        )
    pse_cm = tc.tile_pool(name="psend", bufs=1, space="PSUM")
    pse = pse_cm.__enter__()
    pk2 = pse.tile([128, 2], F32, tag="pk2")
    for oc in range(2):
        for kt in range(2):
            nc.tensor.matmul(
                pk2[:, oc : oc + 1],
                awt[:, kt, R_OUT, oc * 128 : (oc + 1) * 128],
                beta_bf[:, kt : kt + 1],
                start=(kt == 0),
                stop=(kt == 1),
            )
    fbias = stats.tile([128, 2], F32)
    nc.vector.tensor_add(fbias[:], pk2[:], biasv[:, :, B_OUT])

    for oc in range(2):
        po = pse.tile([128, 1024], F32, tag="fin", name=f"pfin_{oc}")
        for half in range(2):
            for kt in range(2):
                nc.tensor.matmul(
                    po[:, half * 512 : (half + 1) * 512],
                    awt_outs[:, kt, oc * 128 : (oc + 1) * 128],
                    rbf[:, kt, half * 512 : (half + 1) * 512],
                    start=(kt == 0),
                    stop=(kt == 1),
                )
        nc.scalar.activation(
            osb[:, oc], po[:], AF.Identity, bias=fbias[:, oc : oc + 1]
        )
    nc.gpsimd.dma_start(out_d.rearrange("(oc p) n -> p oc n", p=128), osb[:])

    pse_cm.__exit__(None, None, None)
    ctx.close()


def _prep_inputs(inputs):
    bf = ml_dtypes.bfloat16
    e4 = ml_dtypes.float8_e4m3
    d = {k: np.asarray(v, dtype=np.float64) for k, v in inputs.items()}

    def conv_wt(w, cin):
        a = w.transpose(1, 2, 3, 0).reshape(cin, 9, 256)
        return np.ascontiguousarray(
            a.reshape(cin // 128, 128, 9, 256).transpose(1, 0, 2, 3)
        ).astype(bf)

    # conv1 weights: fp8 DoubleRow layout [p, ktpair, j, off, out], x16
    a = (d["fs_w1"] * 16.0).transpose(1, 2, 3, 0).reshape(512, 9, 256)
    w1t = np.ascontiguousarray(
        a.reshape(2, 2, 128, 9, 256).transpose(2, 0, 1, 3, 4)
    ).astype(e4)
    w2t = conv_wt(d["fs_w2"], 256)

    # folded attention weights, [cin(style or content), 257] each
    rows = []
    for a in ("a1", "a2"):
        fw, gw = d[f"{a}_fw"], d[f"{a}_gw"]
        v = np.zeros((256, 257))
        v[:, :256] = fw.T @ gw  # lhsT of the query projection (V = gw.T@fw)
        rows.append(v)
    for a in ("a1", "a2"):
        ow, hw, gw, fb = d[f"{a}_ow"], d[f"{a}_hw"], d[f"{a}_gw"], d[f"{a}_fb"]
        blk = np.zeros((256, 257))
        blk[:, :256] = hw.T @ ow.T  # (W_o W_h)^T  [style_c, out_c]
        blk[:, 256] = gw.T @ fb  # per-key softmax bias column
        rows.append(blk)
    outr = np.zeros((256, 257))
    outr[:, :256] = d["out_w"].T
    rows.append(outr)
    awt = np.stack(rows, axis=1)  # [cin, 5, 257]
    awt = np.ascontiguousarray(
        awt.reshape(2, 128, 5, 257).transpose(1, 0, 2, 3)
    ).astype(bf)

    rbias = (
        d["a1_ow"] @ d["a1_hb"] + d["a1_ob"] + d["a2_ow"] @ d["a2_hb"] + d["a2_ob"]
    )
    bvec = np.stack([d["fs_b1"], d["fs_b2"], rbias, d["out_b"]], axis=1)  # [256, 4]
    biasv = np.ascontiguousarray(bvec.reshape(2, 128, 4).transpose(1, 0, 2)).astype(
        np.float32
    )

    xclip_pad = np.zeros((B, CLIP, H + 4, W + 4), np.float32)
    xclip_pad[:, :, 2 : H + 2, 1 : W + 1] = d["F_clip_s"].astype(np.float32)
    cont_f = d["F_content"].astype(np.float32)

    in_maps = []
    for core in range(NCORES):
        b, q = core // 4, core % 4
        xc = xclip_pad[b, :, 16 * q : 16 * q + 20, :]  # [512, 20, 68]
        xc = np.ascontiguousarray(
            xc.reshape(2, 2, 128, 20, 68).transpose(2, 0, 1, 3, 4)
        ).astype(e4)
        cont = cont_f[b].reshape(256, N)
        cont = np.roll(cont, -q * NSLAB, axis=1)
        cont = np.ascontiguousarray(cont.reshape(2, 128, N).transpose(1, 0, 2))
        m = np.ones((128, 18, 1), np.float32)
        if q == 0:
            m[:, 0] = 0.0
        if q == 3:
            m[:, 17] = 0.0
        in_maps.append(
            {
                "xclip": xc,
                "xcont": cont,
                "maskio": m.astype(bf),
                "w1t": w1t,
                "w2t": w2t,
                "awt": awt,
                "biasv": biasv,
            }
        )
    return in_maps


def kernel(**inputs) -> np.ndarray:
    if "nc" not in _CACHE:
        _CACHE["nc"] = _build()
    nc = _CACHE["nc"]
    in_maps = _prep_inputs(inputs)
    res = run_bass_kernel_spmd(nc, in_maps, core_ids=list(range(NCORES)))
    out = np.zeros((B, C, H, W), np.float32)
    for core in range(NCORES):
        b, q = core // 4, core % 4
        out[b, :, 16 * q : 16 * q + 16, :] = res.results[core]["out"].reshape(
            C, ROWS, W
        )
    return out
